# revision 4
# baseline (speedup 1.0000x reference)
"""Biaffine kernel for Trainium2, data-parallel over batch on 8 NeuronCores.

Problem: inputs [8,512,768] f32, weight1 [768,12,768], weight2 [1537,12],
mask [8,512] i32 -> logits [8,12,512,512] f32 (see reference).

Per core (batch b):
  XmT[i,x]   = X[x,i]*m[x]                       (mask + PE transpose)
  tmpT[j,x]  = sum_i W1[i,o,j] * XmT[i,x]        (matmul1, per o)
  raw[x,y]   = sum_j tmpT[j,x] * XmT[j,y]        (matmul2, per o)
               + mx[x]*linjT[o,y] + linT[o,x]*my[y]   (K=2 aug matmul)
  out[o,x,y] = raw + NEG*mxmy[x,y] + C0[x,y]     (one DVE op)
where linT[o,x] = mx[x]*(lin_i[x,o]+bias[o]), linjT[o,y] = my[y]*lin_j[y,o],
C0 = -NEG*tril(k=-1) - NEG.  Masked entries come out bit-exact (-NEG/-2NEG)
because masked matmul inputs are exactly zero.
"""

import numpy as np

import concourse.bass as bass
import concourse.mybir as mybir
import concourse.tile as tile
from concourse import bacc
from concourse.bass_utils import run_bass_kernel_spmd

B, L, H, O = 8, 512, 768, 12
NEG = 1e12
F32 = mybir.dt.float32
F32R = mybir.dt.float32r
NCORES = 8

_cached_nc = None


def build_nc():
    nc = bacc.Bacc(None, target_bir_lowering=False)

    x_d = nc.dram_tensor("x", [L, H], F32R, kind="ExternalInput")
    w1_d = nc.dram_tensor("w1", [H, O, H], F32R, kind="ExternalInput")
    w2a_d = nc.dram_tensor("w2a", [H + 1, 4 * O], F32R, kind="ExternalInput")
    sel_d = nc.dram_tensor("sel", [4 * O, 4 * O], F32R, kind="ExternalInput")
    mrow_d = nc.dram_tensor("mrow", [1, L], F32R, kind="ExternalInput")
    mcol_d = nc.dram_tensor("mcol", [L, 1], F32, kind="ExternalInput")
    ident_d = nc.dram_tensor("ident", [128, 128], F32R, kind="ExternalInput")
    c0_d = nc.dram_tensor("c0", [L, L], F32, kind="ExternalInput")
    out_d = nc.dram_tensor("out", [O, L, L], F32, kind="ExternalOutput")

    KT = H // 128   # 6 k-tiles over i/j
    XC = L // 128   # 4 x-chunks

    with tile.TileContext(nc) as tc:
        with (
            tc.tile_pool(name="const", bufs=1) as cpool,
            tc.tile_pool(name="work", bufs=1) as wpool,
            tc.tile_pool(name="w1p", bufs=2) as w1pool,
            tc.tile_pool(name="tmpp", bufs=2) as tmppool,
            tc.tile_pool(name="augp", bufs=2) as augpool,
            tc.tile_pool(name="outp", bufs=4) as outpool,
        ):
            ident = cpool.tile([128, 128], F32R, tag="ident")
            nc.sync.dma_start(ident[:], ident_d[:])
            mrow = cpool.tile([1, L], F32R, tag="mrow")
            nc.sync.dma_start(mrow[:], mrow_d[:])
            mcol = cpool.tile([128, XC], F32, tag="mcol")
            for c in range(XC):
                nc.sync.dma_start(mcol[:, c : c + 1], mcol_d[c * 128 : (c + 1) * 128, :])
            w2sb = cpool.tile([128, KT * 4 * O], F32R, tag="w2sb")
            for kt in range(KT):
                nc.sync.dma_start(
                    w2sb[:, kt * 4 * O : (kt + 1) * 4 * O],
                    w2a_d[kt * 128 : (kt + 1) * 128, :],
                )
            w2last = cpool.tile([1, 4 * O], F32R, tag="w2last")
            nc.sync.dma_start(w2last[:], w2a_d[H : H + 1, :])
            sel = cpool.tile([4 * O, 4 * O], F32R, tag="sel")
            nc.sync.dma_start(sel[:], sel_d[:])
            csb = cpool.tile([128, XC * L], F32, tag="csb")
            for c in range(XC):
                nc.sync.dma_start(
                    csb[:, c * L : (c + 1) * L], c0_d[c * 128 : (c + 1) * 128, :]
                )

            # --- X natural load + mask + transpose to XmT ---
            xnat = wpool.tile([128, XC * H], F32R, tag="xnat")
            for c in range(XC):
                nc.sync.dma_start(
                    xnat[:, c * H : (c + 1) * H], x_d[c * 128 : (c + 1) * 128, :]
                )
            for c in range(XC):
                nc.vector.tensor_scalar_mul(
                    xnat[:, c * H : (c + 1) * H],
                    xnat[:, c * H : (c + 1) * H],
                    mcol[:, c : c + 1],
                )
            xt = wpool.tile([128, KT * L], F32R, tag="xt")
            with tc.tile_pool(name="pspro", bufs=1, space="PSUM") as pspro:
                for kt in range(KT):
                    for c in range(XC):
                        tp = pspro.tile([128, 128], F32R, tag="tp", bufs=2)
                        nc.tensor.transpose(
                            tp[:],
                            xnat[:, c * H + kt * 128 : c * H + (kt + 1) * 128],
                            ident[:],
                        )
                        nc.vector.tensor_copy(
                            xt[:, kt * L + c * 128 : kt * L + (c + 1) * 128], tp[:]
                        )

                # --- C map: csb <- NEG * (mx outer my) + csb ---
                for c in range(XC):
                    pm = pspro.tile([128, L], F32, tag="pm", bufs=2)
                    nc.tensor.matmul(
                        pm[:],
                        mrow[:, c * 128 : (c + 1) * 128],
                        mrow[:],
                        start=True,
                        stop=True,
                    )
                    nc.vector.scalar_tensor_tensor(
                        out=csb[:, c * L : (c + 1) * L],
                        in0=pm[:],
                        scalar=NEG,
                        in1=csb[:, c * L : (c + 1) * L],
                        op0=mybir.AluOpType.mult,
                        op1=mybir.AluOpType.add,
                    )

                # --- augall [48, 512] = w2a.T @ [XmT; mx] ---
                pa = pspro.tile([4 * O, L], F32, tag="pa", bufs=1)
                for kt in range(KT):
                    nc.tensor.matmul(
                        pa[:],
                        w2sb[:, kt * 4 * O : (kt + 1) * 4 * O],
                        xt[:, kt * L : (kt + 1) * L],
                        start=(kt == 0),
                        stop=False,
                    )
                nc.tensor.matmul(
                    pa[:], w2last[:], mrow[:], start=False, stop=True
                )
                augall = wpool.tile([4 * O, L], F32R, tag="augall")
                nc.vector.tensor_copy(augall[:], pa[:])

            # --- main loop over labels ---
            with tc.tile_pool(name="psmain", bufs=1, space="PSUM") as psm:
                for o in range(O):
                    # per-o aug rows via selection matmuls (base partition 0)
                    pl = psm.tile([2, L], F32, tag="psel", bufs=2)
                    nc.tensor.matmul(
                        pl[:], sel[:, 2 * o : 2 * o + 2], augall[:],
                        start=True, stop=True,
                    )
                    augL = augpool.tile([2, L], F32R, tag="augL")
                    nc.vector.tensor_copy(augL[:], pl[:])
                    pr = psm.tile([2, L], F32, tag="psel", bufs=2)
                    nc.tensor.matmul(
                        pr[:], sel[:, 2 * O + 2 * o : 2 * O + 2 * o + 2], augall[:],
                        start=True, stop=True,
                    )
                    augR = augpool.tile([2, L], F32R, tag="augR")
                    nc.vector.tensor_copy(augR[:], pr[:])

                    # W1[:, o, :] -> SBUF
                    w1t = w1pool.tile([128, KT * H], F32R, tag="w1t")
                    for kt in range(KT):
                        nc.sync.dma_start(
                            w1t[:, kt * H : (kt + 1) * H],
                            w1_d[kt * 128 : (kt + 1) * 128, o, :],
                        )

                    # matmul1: tmpT[j, x] (6 m-chunks x 6 k-tiles)
                    tmp = tmppool.tile([128, KT * L], F32R, tag="tmp")
                    for m in range(KT):
                        p1 = psm.tile([128, L], F32, tag="t1", bufs=2)
                        for kt in range(KT):
                            nc.tensor.matmul(
                                p1[:],
                                w1t[:, kt * H + m * 128 : kt * H + (m + 1) * 128],
                                xt[:, kt * L : (kt + 1) * L],
                                start=(kt == 0),
                                stop=(kt == KT - 1),
                            )
                        nc.vector.tensor_copy(tmp[:, m * L : (m + 1) * L], p1[:])

                    # matmul2 + aug + epilogue per x-chunk
                    for c in range(XC):
                        p2 = psm.tile([128, L], F32, tag="t2", bufs=4)
                        for jr in range(KT):
                            nc.tensor.matmul(
                                p2[:],
                                tmp[:, jr * L + c * 128 : jr * L + (c + 1) * 128],
                                xt[:, jr * L : (jr + 1) * L],
                                start=(jr == 0),
                                stop=False,
                            )
                        nc.tensor.matmul(
                            p2[:],
                            augL[:, c * 128 : (c + 1) * 128],
                            augR[:],
                            start=False,
                            stop=True,
                        )
                        osb = outpool.tile([128, L], F32, tag="osb")
                        nc.vector.tensor_add(
                            osb[:], p2[:], csb[:, c * L : (c + 1) * L]
                        )
                        nc.sync.dma_start(
                            out_d[o, c * 128 : (c + 1) * 128, :], osb[:]
                        )

    nc.compile()
    return nc


def _get_nc():
    global _cached_nc
    if _cached_nc is None:
        _cached_nc = build_nc()
    return _cached_nc


def _host_consts(weight1, weight2):
    w2a = np.zeros((H + 1, 4 * O), dtype=np.float32)
    for o in range(O):
        # lhsT aug pair: row0 = mx (selector e_H), row1 = linT = mx*(lin_i+bias)
        w2a[H, 2 * o] = 1.0
        w2a[:H, 2 * o + 1] = weight2[:H, o]
        w2a[H, 2 * o + 1] = weight2[2 * H, o]
        # rhs aug pair: row0 = linjT = my*lin_j, row1 = my (selector e_H)
        w2a[:H, 2 * O + 2 * o] = weight2[H : 2 * H, o]
        w2a[H, 2 * O + 2 * o + 1] = 1.0
    sel = np.eye(4 * O, dtype=np.float32)
    ident = np.eye(128, dtype=np.float32)
    tril = np.tril(np.ones((L, L), dtype=np.float32), k=-1)
    c0 = (-NEG * tril - NEG).astype(np.float32)
    return w2a, sel, ident, c0


def _run(inputs, weight1, weight2, mask, trace=False):
    nc = _get_nc()
    w2a, sel, ident, c0 = _host_consts(
        np.asarray(weight1, dtype=np.float32), np.asarray(weight2, dtype=np.float32)
    )
    w1 = np.ascontiguousarray(np.asarray(weight1, dtype=np.float32))
    in_maps = []
    for b in range(NCORES):
        m = np.asarray(mask[b], dtype=np.float32)
        in_maps.append(
            {
                "x": np.ascontiguousarray(np.asarray(inputs[b], dtype=np.float32)),
                "w1": w1,
                "w2a": w2a,
                "sel": sel,
                "mrow": np.ascontiguousarray(m[None, :]),
                "mcol": np.ascontiguousarray(m[:, None]),
                "ident": ident,
                "c0": c0,
            }
        )
    br = run_bass_kernel_spmd(nc, in_maps, core_ids=list(range(NCORES)), trace=trace)
    out = np.stack([br.results[b]["out"] for b in range(NCORES)], axis=0)
    return out, br


def kernel(inputs, weight1, weight2, mask):
    out, _ = _run(inputs, weight1, weight2, mask)
    return out


# revision 5
# speedup vs baseline: 1.0340x; 1.0340x over previous
"""Biaffine kernel for Trainium2, data-parallel over batch on 8 NeuronCores.

Problem: inputs [8,512,768] f32, weight1 [768,12,768], weight2 [1537,12],
mask [8,512] i32 -> logits [8,12,512,512] f32 (see reference).

Key trick: masked outputs are -1e12 (or -2e12), and f32 addition absorbs any
|v| < half-ulp(1e12) = 32768. Raw logits are |v| <~ 1e3, so we never mask the
matmul inputs: out = raw + C with C in {0, -1e12, -2e12} reproduces the
reference bit-exactly on masked entries and exactly on unmasked ones.

Per core (batch b):
  XT[i,x]    = X[x,i]                             (PE transpose)
  tmpT[j,x]  = sum_i W1[i,o,j] * XT[i,x]          (matmul1, per o)
  raw[x,y]   = sum_j tmpT[j,x] * XT[j,y]          (matmul2, per o)
               + linjT[o,y]                       (K=24 selector matmul)
  out[o,x,y] = (raw + linT[o,x]) + C[x,y]         (one DVE scalar_tensor_tensor)
where linT[o,x] = lin_i[x,o]+bias[o], linjT[o,y] = lin_j[y,o],
C = NEG*(m outer m) + C0,  C0 = -NEG*tril(k=-1) - NEG.
"""

import numpy as np

import concourse.bass as bass
import concourse.mybir as mybir
import concourse.tile as tile
from concourse import bacc
from concourse.bass_utils import run_bass_kernel_spmd

B, L, H, O = 8, 512, 768, 12
NEG = 1e12
F32 = mybir.dt.float32
F32R = mybir.dt.float32r
NCORES = 8

_cached_nc = None


def build_nc():
    nc = bacc.Bacc(None, target_bir_lowering=False)

    x_d = nc.dram_tensor("x", [L, H], F32R, kind="ExternalInput")
    w1_d = nc.dram_tensor("w1", [H, O, H], F32R, kind="ExternalInput")
    w2a_d = nc.dram_tensor("w2a", [H + 1, 2 * O], F32R, kind="ExternalInput")
    selo_d = nc.dram_tensor("selo", [2 * O, O * 128], F32R, kind="ExternalInput")
    mrow_d = nc.dram_tensor("mrow", [1, L], F32R, kind="ExternalInput")
    ones_d = nc.dram_tensor("ones1", [1, L], F32R, kind="ExternalInput")
    ident_d = nc.dram_tensor("ident", [128, 128], F32R, kind="ExternalInput")
    c0_d = nc.dram_tensor("c0", [L, L], F32, kind="ExternalInput")
    out_d = nc.dram_tensor("out", [O, L, L], F32, kind="ExternalOutput")

    KT = H // 128   # 6 k-tiles over i/j
    XC = L // 128   # 4 x-chunks

    with tile.TileContext(nc) as tc:
        with (
            tc.tile_pool(name="const", bufs=1) as cpool,
            tc.tile_pool(name="work", bufs=1) as wpool,
            tc.tile_pool(name="w1p", bufs=2) as w1pool,
            tc.tile_pool(name="tmpp", bufs=2) as tmppool,
            tc.tile_pool(name="outp", bufs=4) as outpool,
        ):
            ident = cpool.tile([128, 128], F32R, tag="ident")
            nc.sync.dma_start(ident[:], ident_d[:])
            mrow = cpool.tile([1, L], F32R, tag="mrow")
            nc.sync.dma_start(mrow[:], mrow_d[:])
            ones1 = cpool.tile([1, L], F32R, tag="ones1")
            nc.sync.dma_start(ones1[:], ones_d[:])
            w2sb = cpool.tile([128, KT * 2 * O], F32R, tag="w2sb")
            for kt in range(KT):
                nc.sync.dma_start(
                    w2sb[:, kt * 2 * O : (kt + 1) * 2 * O],
                    w2a_d[kt * 128 : (kt + 1) * 128, :],
                )
            w2last = cpool.tile([1, 2 * O], F32R, tag="w2last")
            nc.sync.dma_start(w2last[:], w2a_d[H : H + 1, :])
            selo = cpool.tile([2 * O, O * 128], F32R, tag="selo")
            nc.sync.dma_start(selo[:], selo_d[:])
            csb = cpool.tile([128, XC * L], F32, tag="csb")
            for c in range(XC):
                nc.sync.dma_start(
                    csb[:, c * L : (c + 1) * L], c0_d[c * 128 : (c + 1) * 128, :]
                )

            # --- X natural load + transpose to XT ---
            xnat = wpool.tile([128, XC * H], F32R, tag="xnat")
            for c in range(XC):
                nc.sync.dma_start(
                    xnat[:, c * H : (c + 1) * H], x_d[c * 128 : (c + 1) * 128, :]
                )
            xt = wpool.tile([128, KT * L], F32R, tag="xt")
            augall = wpool.tile([2 * O, L], F32R, tag="augall")
            linTT = wpool.tile([128, XC * O], F32, tag="linTT")
            with tc.tile_pool(name="pspro", bufs=1, space="PSUM") as pspro:
                for kt in range(KT):
                    for c in range(XC):
                        tp = pspro.tile([128, 128], F32R, tag="tp", bufs=2)
                        nc.tensor.transpose(
                            tp[:],
                            xnat[:, c * H + kt * 128 : c * H + (kt + 1) * 128],
                            ident[:],
                        )
                        nc.vector.tensor_copy(
                            xt[:, kt * L + c * 128 : kt * L + (c + 1) * 128], tp[:]
                        )

                # --- C map: csb <- NEG * (mx outer my) + csb ---
                for c in range(XC):
                    pm = pspro.tile([128, L], F32, tag="pm", bufs=2)
                    nc.tensor.matmul(
                        pm[:],
                        mrow[:, c * 128 : (c + 1) * 128],
                        mrow[:],
                        start=True,
                        stop=True,
                    )
                    nc.vector.scalar_tensor_tensor(
                        out=csb[:, c * L : (c + 1) * L],
                        in0=pm[:],
                        scalar=NEG,
                        in1=csb[:, c * L : (c + 1) * L],
                        op0=mybir.AluOpType.mult,
                        op1=mybir.AluOpType.add,
                    )

                # --- augall [24, 512]: rows o = linT'[o], rows 12+o = linjT'[o]
                pa = pspro.tile([2 * O, L], F32, tag="pa", bufs=1)
                for kt in range(KT):
                    nc.tensor.matmul(
                        pa[:],
                        w2sb[:, kt * 2 * O : (kt + 1) * 2 * O],
                        xt[:, kt * L : (kt + 1) * L],
                        start=(kt == 0),
                        stop=False,
                    )
                nc.tensor.matmul(
                    pa[:], w2last[:], ones1[:], start=False, stop=True
                )
                nc.vector.tensor_copy(augall[:], pa[:])

                # --- linTT [128, XC*O]: transpose of augall rows 0..11 ---
                for c in range(XC):
                    pt = pspro.tile([128, O], F32R, tag="pt", bufs=2)
                    nc.tensor.transpose(
                        pt[:],
                        augall[0:O, c * 128 : (c + 1) * 128],
                        ident[0:O, 0:O],
                    )
                    nc.vector.tensor_copy(linTT[:, c * O : (c + 1) * O], pt[:])

            # --- main loop over labels ---
            with tc.tile_pool(name="psmain", bufs=1, space="PSUM") as psm:
                for o in range(O):
                    # W1[:, o, :] -> SBUF
                    w1t = w1pool.tile([128, KT * H], F32R, tag="w1t")
                    for kt in range(KT):
                        nc.sync.dma_start(
                            w1t[:, kt * H : (kt + 1) * H],
                            w1_d[kt * 128 : (kt + 1) * 128, o, :],
                        )

                    # matmul1: tmpT[j, x] (6 m-chunks x 6 k-tiles)
                    tmp = tmppool.tile([128, KT * L], F32R, tag="tmp")
                    for m in range(KT):
                        p1 = psm.tile([128, L], F32, tag="t1", bufs=3)
                        for kt in range(KT):
                            nc.tensor.matmul(
                                p1[:],
                                w1t[:, kt * H + m * 128 : kt * H + (m + 1) * 128],
                                xt[:, kt * L : (kt + 1) * L],
                                start=(kt == 0),
                                stop=(kt == KT - 1),
                            )
                        nc.vector.tensor_copy(tmp[:, m * L : (m + 1) * L], p1[:])

                    # matmul2 + linj aug + epilogue per x-chunk
                    for c in range(XC):
                        p2 = psm.tile([128, L], F32, tag="t2", bufs=5)
                        for jr in range(KT):
                            nc.tensor.matmul(
                                p2[:],
                                tmp[:, jr * L + c * 128 : jr * L + (c + 1) * 128],
                                xt[:, jr * L : (jr + 1) * L],
                                start=(jr == 0),
                                stop=False,
                            )
                        nc.tensor.matmul(
                            p2[:],
                            selo[:, o * 128 : (o + 1) * 128],
                            augall[:],
                            start=False,
                            stop=True,
                        )
                        osb = outpool.tile([128, L], F32, tag="osb")
                        nc.vector.scalar_tensor_tensor(
                            out=osb[:],
                            in0=p2[:],
                            scalar=linTT[:, c * O + o : c * O + o + 1],
                            in1=csb[:, c * L : (c + 1) * L],
                            op0=mybir.AluOpType.add,
                            op1=mybir.AluOpType.add,
                        )
                        nc.sync.dma_start(
                            out_d[o, c * 128 : (c + 1) * 128, :], osb[:]
                        )

    nc.compile()
    return nc


def _get_nc():
    global _cached_nc
    if _cached_nc is None:
        _cached_nc = build_nc()
    return _cached_nc


def _host_consts(weight2):
    w2a = np.zeros((H + 1, 2 * O), dtype=np.float32)
    # cols o: linT' = lin_i + bias; cols O+o: linjT' = lin_j
    w2a[:H, :O] = weight2[:H, :]
    w2a[H, :O] = weight2[2 * H, :]
    w2a[:H, O : 2 * O] = weight2[H : 2 * H, :]
    selo = np.zeros((2 * O, O * 128), dtype=np.float32)
    for o in range(O):
        selo[O + o, o * 128 : (o + 1) * 128] = 1.0
    ident = np.eye(128, dtype=np.float32)
    ones1 = np.ones((1, L), dtype=np.float32)
    tril = np.tril(np.ones((L, L), dtype=np.float32), k=-1)
    c0 = (-NEG * tril - NEG).astype(np.float32)
    return w2a, selo, ident, ones1, c0


def _run(inputs, weight1, weight2, mask, trace=False):
    nc = _get_nc()
    w2a, selo, ident, ones1, c0 = _host_consts(np.asarray(weight2, dtype=np.float32))
    w1 = np.ascontiguousarray(np.asarray(weight1, dtype=np.float32))
    in_maps = []
    for b in range(NCORES):
        m = np.asarray(mask[b], dtype=np.float32)
        in_maps.append(
            {
                "x": np.ascontiguousarray(np.asarray(inputs[b], dtype=np.float32)),
                "w1": w1,
                "w2a": w2a,
                "selo": selo,
                "mrow": np.ascontiguousarray(m[None, :]),
                "ones1": ones1,
                "ident": ident,
                "c0": c0,
            }
        )
    br = run_bass_kernel_spmd(nc, in_maps, core_ids=list(range(NCORES)), trace=trace)
    out = np.stack([br.results[b]["out"] for b in range(NCORES)], axis=0)
    return out, br


def kernel(inputs, weight1, weight2, mask):
    out, _ = _run(inputs, weight1, weight2, mask)
    return out


# revision 8
# speedup vs baseline: 1.1390x; 1.1016x over previous
"""Biaffine kernel for Trainium2, data-parallel over batch on 8 NeuronCores.

Problem: inputs [8,512,768] f32, weight1 [768,12,768], weight2 [1537,12],
mask [8,512] i32 -> logits [8,12,512,512] f32 (see reference).

Key trick: masked outputs are -1e12 (or -2e12), and f32 addition absorbs any
|v| < half-ulp(1e12) = 32768. Raw logits are |v| <~ 1e3, so we never mask the
matmul inputs: out = raw + C with C in {0, -1e12, -2e12} reproduces the
reference bit-exactly on masked entries and exactly on unmasked ones.

Per core (batch b):
  XT[i,x]    = X[x,i]                             (PE transpose)
  tmpT[j,x]  = sum_i W1[i,o,j] * XT[i,x]          (matmul1, per o)
  raw[x,y]   = sum_j tmpT[j,x] * XT[j,y]          (matmul2, per o)
               + linjT[o,y]                       (K=24 selector matmul)
  out[o,x,y] = (raw + linT[o,x]) + C[x,y]         (one DVE scalar_tensor_tensor)
where linT[o,x] = lin_i[x,o]+bias[o], linjT[o,y] = lin_j[y,o],
C = NEG*(m outer m) + C0,  C0 = -NEG*tril(k=-1) - NEG.
"""

import numpy as np

import concourse.bass as bass
import concourse.mybir as mybir
import concourse.tile as tile
from concourse import bacc
from concourse.bass_utils import run_bass_kernel_spmd

B, L, H, O = 8, 512, 768, 12
NEG = 1e12
F32 = mybir.dt.float32
F32R = mybir.dt.float32r
NCORES = 8

_cached_nc = None


def build_nc():
    nc = bacc.Bacc(None, target_bir_lowering=False)

    x_d = nc.dram_tensor("x", [L, H], F32R, kind="ExternalInput")
    w1_d = nc.dram_tensor("w1", [H, O, H], F32R, kind="ExternalInput")
    w2a_d = nc.dram_tensor("w2a", [H + 1, 128], F32R, kind="ExternalInput")
    selo_d = nc.dram_tensor("selo", [128, O * 128], F32R, kind="ExternalInput")
    mrow_d = nc.dram_tensor("mrow", [1, L], F32R, kind="ExternalInput")
    ones_d = nc.dram_tensor("ones1", [1, L], F32R, kind="ExternalInput")
    ident_d = nc.dram_tensor("ident", [128, 128], F32R, kind="ExternalInput")
    c0_d = nc.dram_tensor("c0", [L, L], F32, kind="ExternalInput")
    out_d = nc.dram_tensor("out", [O, L, L], F32, kind="ExternalOutput")

    KT = H // 128   # 6 k-tiles over i/j
    XC = L // 128   # 4 x-chunks

    with tile.TileContext(nc) as tc:
        with (
            tc.tile_pool(name="const", bufs=1) as cpool,
            tc.tile_pool(name="work", bufs=1) as wpool,
            tc.tile_pool(name="w1p", bufs=2) as w1pool,
            tc.tile_pool(name="tmpp", bufs=2) as tmppool,
            tc.tile_pool(name="outp", bufs=4) as outpool,
        ):
            # X first (transposes gate everything), then W1[o=0] prefetch,
            # then the other constants.
            xnat = wpool.tile([128, XC * H], F32R, tag="xnat")
            for c in range(XC):
                nc.sync.dma_start(
                    xnat[:, c * H : (c + 1) * H], x_d[c * 128 : (c + 1) * 128, :]
                )
            ident = cpool.tile([128, 128], F32R, tag="ident")
            nc.sync.dma_start(ident[:], ident_d[:])
            w1t_next = w1pool.tile([128, KT * H], F32R, tag="w1t")
            for kt in range(KT):
                nc.sync.dma_start(
                    w1t_next[:, kt * H : (kt + 1) * H],
                    w1_d[kt * 128 : (kt + 1) * 128, 0, :],
                )
            mrow = cpool.tile([1, L], F32R, tag="mrow")
            nc.sync.dma_start(mrow[:], mrow_d[:])
            ones1 = cpool.tile([1, L], F32R, tag="ones1")
            nc.sync.dma_start(ones1[:], ones_d[:])
            w2sb = cpool.tile([128, KT * 128], F32R, tag="w2sb")
            for kt in range(KT):
                nc.sync.dma_start(
                    w2sb[:, kt * 128 : (kt + 1) * 128],
                    w2a_d[kt * 128 : (kt + 1) * 128, :],
                )
            w2last = cpool.tile([1, 128], F32R, tag="w2last")
            nc.sync.dma_start(w2last[:], w2a_d[H : H + 1, :])
            selo = cpool.tile([128, O * 128], F32R, tag="selo")
            nc.sync.dma_start(selo[:], selo_d[:])
            csb = cpool.tile([128, XC * L], F32, tag="csb")
            for c in range(XC):
                nc.sync.dma_start(
                    csb[:, c * L : (c + 1) * L], c0_d[c * 128 : (c + 1) * 128, :]
                )
            xt = wpool.tile([128, KT * L], F32R, tag="xt")
            augall = wpool.tile([128, L], F32R, tag="augall")
            linTT = wpool.tile([128, XC * O], F32, tag="linTT")
            with tc.tile_pool(name="pspro", bufs=1, space="PSUM") as pspro:
                for kt in range(KT):
                    for c in range(XC):
                        tp = pspro.tile([128, 128], F32R, tag="tp", bufs=2)
                        nc.tensor.transpose(
                            tp[:],
                            xnat[:, c * H + kt * 128 : c * H + (kt + 1) * 128],
                            ident[:],
                        )
                        nc.vector.tensor_copy(
                            xt[:, kt * L + c * 128 : kt * L + (c + 1) * 128], tp[:]
                        )

                # --- C map: csb <- NEG * (mx outer my) + csb ---
                for c in range(XC):
                    pm = pspro.tile([128, L], F32, tag="pm", bufs=2)
                    nc.tensor.matmul(
                        pm[:],
                        mrow[:, c * 128 : (c + 1) * 128],
                        mrow[:],
                        start=True,
                        stop=True,
                    )
                    nc.vector.scalar_tensor_tensor(
                        out=csb[:, c * L : (c + 1) * L],
                        in0=pm[:],
                        scalar=NEG,
                        in1=csb[:, c * L : (c + 1) * L],
                        op0=mybir.AluOpType.mult,
                        op1=mybir.AluOpType.add,
                    )

                # --- augall [24, 512]: rows o = linT'[o], rows 12+o = linjT'[o]
                pa = pspro.tile([128, L], F32, tag="pa", bufs=1)
                for kt in range(KT):
                    nc.tensor.matmul(
                        pa[:],
                        w2sb[:, kt * 128 : (kt + 1) * 128],
                        xt[:, kt * L : (kt + 1) * L],
                        start=(kt == 0),
                        stop=False,
                    )
                nc.tensor.matmul(
                    pa[:], w2last[:], ones1[:], start=False, stop=True
                )
                nc.vector.tensor_copy(augall[:], pa[:])

                # --- linTT [128, XC*O]: transpose of augall rows 0..11 ---
                for c in range(XC):
                    pt = pspro.tile([128, O], F32R, tag="pt", bufs=2)
                    nc.tensor.transpose(
                        pt[:],
                        augall[0:O, c * 128 : (c + 1) * 128],
                        ident[0:O, 0:O],
                    )
                    nc.vector.tensor_copy(linTT[:, c * O : (c + 1) * O], pt[:])

            # --- main loop over labels ---
            with tc.tile_pool(name="psmain", bufs=1, space="PSUM") as psm:
                for o in range(O):
                    w1t = w1t_next
                    if o + 1 < O:
                        w1t_next = w1pool.tile([128, KT * H], F32R, tag="w1t")
                        for kt in range(KT):
                            nc.sync.dma_start(
                                w1t_next[:, kt * H : (kt + 1) * H],
                                w1_d[kt * 128 : (kt + 1) * 128, o + 1, :],
                            )

                    # matmul1: tmpT[j, x] (6 m-chunks x 6 k-tiles)
                    tmp = tmppool.tile([128, KT * L], F32R, tag="tmp")
                    for m in range(KT):
                        p1 = psm.tile([128, L], F32, tag="t1", bufs=3)
                        for kt in range(KT):
                            nc.tensor.matmul(
                                p1[:],
                                w1t[:, kt * H + m * 128 : kt * H + (m + 1) * 128],
                                xt[:, kt * L : (kt + 1) * L],
                                start=(kt == 0),
                                stop=(kt == KT - 1),
                            )
                        nc.vector.tensor_copy(tmp[:, m * L : (m + 1) * L], p1[:])

                    # matmul2 + linj aug + epilogue per x-chunk
                    for c in range(XC):
                        p2 = psm.tile([128, L], F32, tag="t2", bufs=5)
                        for jr in range(KT):
                            nc.tensor.matmul(
                                p2[:],
                                tmp[:, jr * L + c * 128 : jr * L + (c + 1) * 128],
                                xt[:, jr * L : (jr + 1) * L],
                                start=(jr == 0),
                                stop=False,
                            )
                        nc.tensor.matmul(
                            p2[:],
                            selo[:, o * 128 : (o + 1) * 128],
                            augall[:],
                            start=False,
                            stop=True,
                        )
                        osb = outpool.tile([128, L], F32, tag="osb")
                        nc.vector.scalar_tensor_tensor(
                            out=osb[:],
                            in0=p2[:],
                            scalar=linTT[:, c * O + o : c * O + o + 1],
                            in1=csb[:, c * L : (c + 1) * L],
                            op0=mybir.AluOpType.add,
                            op1=mybir.AluOpType.add,
                        )
                        nc.scalar.dma_start(
                            out_d[o, c * 128 : (c + 1) * 128, :], osb[:]
                        )

    nc.compile()
    return nc


def _get_nc():
    global _cached_nc
    if _cached_nc is None:
        _cached_nc = build_nc()
    return _cached_nc


def _host_consts(weight2):
    w2a = np.zeros((H + 1, 128), dtype=np.float32)
    # cols o: linT' = lin_i + bias; cols O+o: linjT' = lin_j
    w2a[:H, :O] = weight2[:H, :]
    w2a[H, :O] = weight2[2 * H, :]
    w2a[:H, O : 2 * O] = weight2[H : 2 * H, :]
    selo = np.zeros((128, O * 128), dtype=np.float32)
    for o in range(O):
        selo[O + o, o * 128 : (o + 1) * 128] = 1.0
    ident = np.eye(128, dtype=np.float32)
    ones1 = np.ones((1, L), dtype=np.float32)
    tril = np.tril(np.ones((L, L), dtype=np.float32), k=-1)
    c0 = (-NEG * tril - NEG).astype(np.float32)
    return w2a, selo, ident, ones1, c0


def _run(inputs, weight1, weight2, mask, trace=False):
    nc = _get_nc()
    w2a, selo, ident, ones1, c0 = _host_consts(np.asarray(weight2, dtype=np.float32))
    w1 = np.ascontiguousarray(np.asarray(weight1, dtype=np.float32))
    in_maps = []
    for b in range(NCORES):
        m = np.asarray(mask[b], dtype=np.float32)
        in_maps.append(
            {
                "x": np.ascontiguousarray(np.asarray(inputs[b], dtype=np.float32)),
                "w1": w1,
                "w2a": w2a,
                "selo": selo,
                "mrow": np.ascontiguousarray(m[None, :]),
                "ones1": ones1,
                "ident": ident,
                "c0": c0,
            }
        )
    br = run_bass_kernel_spmd(nc, in_maps, core_ids=list(range(NCORES)), trace=trace)
    out = np.stack([br.results[b]["out"] for b in range(NCORES)], axis=0)
    return out, br


def kernel(inputs, weight1, weight2, mask):
    out, _ = _run(inputs, weight1, weight2, mask)
    return out


# revision 9
# speedup vs baseline: 1.1560x; 1.0149x over previous
"""Biaffine kernel for Trainium2, data-parallel over batch on 8 NeuronCores.

Problem: inputs [8,512,768] f32, weight1 [768,12,768], weight2 [1537,12],
mask [8,512] i32 -> logits [8,12,512,512] f32 (see reference).

Key trick: masked outputs are -1e12 (or -2e12), and f32 addition absorbs any
|v| < half-ulp(1e12) = 32768. Raw logits are |v| <~ 1e3, so we never mask the
matmul inputs: out = raw + C with C in {0, -1e12, -2e12} reproduces the
reference bit-exactly on masked entries and exactly on unmasked ones.

Per core (batch b):
  XT[i,x]    = X[x,i]                             (PE transpose)
  tmpT[j,x]  = sum_i W1[i,o,j] * XT[i,x]          (matmul1, per o)
  raw[x,y]   = sum_j tmpT[j,x] * XT[j,y]          (matmul2, per o)
               + linjT[o,y]                       (K=24 selector matmul)
  out[o,x,y] = (raw + linT[o,x]) + C[x,y]         (one DVE scalar_tensor_tensor)
where linT[o,x] = lin_i[x,o]+bias[o], linjT[o,y] = lin_j[y,o],
C = NEG*(m outer m) + C0,  C0 = -NEG*tril(k=-1) - NEG.
"""

import numpy as np

import concourse.bass as bass
import concourse.mybir as mybir
import concourse.tile as tile
from concourse import bacc
from concourse.bass_utils import run_bass_kernel_spmd

B, L, H, O = 8, 512, 768, 12
NEG = 1e12
F32 = mybir.dt.float32
F32R = mybir.dt.float32r
NCORES = 8

_cached_nc = None


def build_nc():
    nc = bacc.Bacc(None, target_bir_lowering=False)

    x_d = nc.dram_tensor("x", [L, H], F32R, kind="ExternalInput")
    w1_d = nc.dram_tensor("w1", [H, O, H], F32R, kind="ExternalInput")
    w2a_d = nc.dram_tensor("w2a", [H + 1, 128], F32R, kind="ExternalInput")
    selo_d = nc.dram_tensor("selo", [128, O * 128], F32R, kind="ExternalInput")
    mrow_d = nc.dram_tensor("mrow", [1, L], F32R, kind="ExternalInput")
    ones_d = nc.dram_tensor("ones1", [1, L], F32R, kind="ExternalInput")
    ident_d = nc.dram_tensor("ident", [128, 128], F32R, kind="ExternalInput")
    c0_d = nc.dram_tensor("c0", [L, L], F32, kind="ExternalInput")
    out_d = nc.dram_tensor("out", [O, L, L], F32, kind="ExternalOutput")

    KT = H // 128   # 6 k-tiles over i/j
    XC = L // 128   # 4 x-chunks

    with tile.TileContext(nc) as tc:
        with (
            tc.tile_pool(name="const", bufs=1) as cpool,
            tc.tile_pool(name="work", bufs=1) as wpool,
            tc.tile_pool(name="w1p", bufs=2) as w1pool,
            tc.tile_pool(name="tmpp", bufs=2) as tmppool,
            tc.tile_pool(name="outp", bufs=4) as outpool,
        ):
            # ident+X first on the sync queue (transposes gate everything);
            # W1[o=0] prefetch goes on the scalar HWDGE queue so it does not
            # delay the transposes' DMA waits.
            ident = cpool.tile([128, 128], F32R, tag="ident")
            nc.sync.dma_start(ident[:], ident_d[:])
            xnat = wpool.tile([128, XC * H], F32R, tag="xnat")
            for c in range(XC):
                nc.sync.dma_start(
                    xnat[:, c * H : (c + 1) * H], x_d[c * 128 : (c + 1) * 128, :]
                )
            w1t_next = w1pool.tile([128, KT * H], F32R, tag="w1t")
            for kt in range(KT):
                nc.scalar.dma_start(
                    w1t_next[:, kt * H : (kt + 1) * H],
                    w1_d[kt * 128 : (kt + 1) * 128, 0, :],
                )
            mrow = cpool.tile([1, L], F32R, tag="mrow")
            nc.sync.dma_start(mrow[:], mrow_d[:])
            ones1 = cpool.tile([1, L], F32R, tag="ones1")
            nc.sync.dma_start(ones1[:], ones_d[:])
            w2sb = cpool.tile([128, KT * 128], F32R, tag="w2sb")
            for kt in range(KT):
                nc.sync.dma_start(
                    w2sb[:, kt * 128 : (kt + 1) * 128],
                    w2a_d[kt * 128 : (kt + 1) * 128, :],
                )
            w2last = cpool.tile([1, 128], F32R, tag="w2last")
            nc.sync.dma_start(w2last[:], w2a_d[H : H + 1, :])
            selo = cpool.tile([128, O * 128], F32R, tag="selo")
            nc.sync.dma_start(selo[:], selo_d[:])
            csb = cpool.tile([128, XC * L], F32, tag="csb")
            for c in range(XC):
                nc.sync.dma_start(
                    csb[:, c * L : (c + 1) * L], c0_d[c * 128 : (c + 1) * 128, :]
                )
            xt = wpool.tile([128, KT * L], F32R, tag="xt")
            augall = wpool.tile([128, L], F32R, tag="augall")
            linTT = wpool.tile([128, XC * O], F32, tag="linTT")
            with tc.tile_pool(name="pspro", bufs=1, space="PSUM") as pspro:
                for c in range(XC):
                    for kt in range(KT):
                        tp = pspro.tile([128, 128], F32R, tag="tp", bufs=2)
                        nc.tensor.transpose(
                            tp[:],
                            xnat[:, c * H + kt * 128 : c * H + (kt + 1) * 128],
                            ident[:],
                        )
                        nc.vector.tensor_copy(
                            xt[:, kt * L + c * 128 : kt * L + (c + 1) * 128], tp[:]
                        )

                # --- C map: csb <- NEG * (mx outer my) + csb ---
                for c in range(XC):
                    pm = pspro.tile([128, L], F32, tag="pm", bufs=2)
                    nc.tensor.matmul(
                        pm[:],
                        mrow[:, c * 128 : (c + 1) * 128],
                        mrow[:],
                        start=True,
                        stop=True,
                    )
                    nc.vector.scalar_tensor_tensor(
                        out=csb[:, c * L : (c + 1) * L],
                        in0=pm[:],
                        scalar=NEG,
                        in1=csb[:, c * L : (c + 1) * L],
                        op0=mybir.AluOpType.mult,
                        op1=mybir.AluOpType.add,
                    )

                # --- augall [24, 512]: rows o = linT'[o], rows 12+o = linjT'[o]
                pa = pspro.tile([128, L], F32, tag="pa", bufs=1)
                for kt in range(KT):
                    nc.tensor.matmul(
                        pa[:],
                        w2sb[:, kt * 128 : (kt + 1) * 128],
                        xt[:, kt * L : (kt + 1) * L],
                        start=(kt == 0),
                        stop=False,
                    )
                nc.tensor.matmul(
                    pa[:], w2last[:], ones1[:], start=False, stop=True
                )
                nc.vector.tensor_copy(augall[:], pa[:])

                # --- linTT [128, XC*O]: transpose of augall rows 0..11 ---
                for c in range(XC):
                    pt = pspro.tile([128, O], F32R, tag="pt", bufs=2)
                    nc.tensor.transpose(
                        pt[:],
                        augall[0:O, c * 128 : (c + 1) * 128],
                        ident[0:O, 0:O],
                    )
                    nc.vector.tensor_copy(linTT[:, c * O : (c + 1) * O], pt[:])

            # --- main loop over labels ---
            with tc.tile_pool(name="psmain", bufs=1, space="PSUM") as psm:
                for o in range(O):
                    w1t = w1t_next
                    if o + 1 < O:
                        w1t_next = w1pool.tile([128, KT * H], F32R, tag="w1t")
                        for kt in range(KT):
                            nc.sync.dma_start(
                                w1t_next[:, kt * H : (kt + 1) * H],
                                w1_d[kt * 128 : (kt + 1) * 128, o + 1, :],
                            )

                    # matmul1: tmpT[j, x] (6 m-chunks x 6 k-tiles)
                    tmp = tmppool.tile([128, KT * L], F32R, tag="tmp")
                    for m in range(KT):
                        p1 = psm.tile([128, L], F32, tag="t1", bufs=3)
                        for kt in range(KT):
                            nc.tensor.matmul(
                                p1[:],
                                w1t[:, kt * H + m * 128 : kt * H + (m + 1) * 128],
                                xt[:, kt * L : (kt + 1) * L],
                                start=(kt == 0),
                                stop=(kt == KT - 1),
                            )
                        nc.vector.tensor_copy(tmp[:, m * L : (m + 1) * L], p1[:])

                    # matmul2 + linj aug + epilogue per x-chunk
                    for c in range(XC):
                        p2 = psm.tile([128, L], F32, tag="t2", bufs=5)
                        for jr in range(KT):
                            nc.tensor.matmul(
                                p2[:],
                                tmp[:, jr * L + c * 128 : jr * L + (c + 1) * 128],
                                xt[:, jr * L : (jr + 1) * L],
                                start=(jr == 0),
                                stop=False,
                            )
                        nc.tensor.matmul(
                            p2[:],
                            selo[:, o * 128 : (o + 1) * 128],
                            augall[:],
                            start=False,
                            stop=True,
                        )
                        osb = outpool.tile([128, L], F32, tag="osb")
                        nc.vector.scalar_tensor_tensor(
                            out=osb[:],
                            in0=p2[:],
                            scalar=linTT[:, c * O + o : c * O + o + 1],
                            in1=csb[:, c * L : (c + 1) * L],
                            op0=mybir.AluOpType.add,
                            op1=mybir.AluOpType.add,
                        )
                        nc.scalar.dma_start(
                            out_d[o, c * 128 : (c + 1) * 128, :], osb[:]
                        )

    nc.compile()
    return nc


def _get_nc():
    global _cached_nc
    if _cached_nc is None:
        _cached_nc = build_nc()
    return _cached_nc


def _host_consts(weight2):
    w2a = np.zeros((H + 1, 128), dtype=np.float32)
    # cols o: linT' = lin_i + bias; cols O+o: linjT' = lin_j
    w2a[:H, :O] = weight2[:H, :]
    w2a[H, :O] = weight2[2 * H, :]
    w2a[:H, O : 2 * O] = weight2[H : 2 * H, :]
    selo = np.zeros((128, O * 128), dtype=np.float32)
    for o in range(O):
        selo[O + o, o * 128 : (o + 1) * 128] = 1.0
    ident = np.eye(128, dtype=np.float32)
    ones1 = np.ones((1, L), dtype=np.float32)
    tril = np.tril(np.ones((L, L), dtype=np.float32), k=-1)
    c0 = (-NEG * tril - NEG).astype(np.float32)
    return w2a, selo, ident, ones1, c0


def _run(inputs, weight1, weight2, mask, trace=False):
    nc = _get_nc()
    w2a, selo, ident, ones1, c0 = _host_consts(np.asarray(weight2, dtype=np.float32))
    w1 = np.ascontiguousarray(np.asarray(weight1, dtype=np.float32))
    in_maps = []
    for b in range(NCORES):
        m = np.asarray(mask[b], dtype=np.float32)
        in_maps.append(
            {
                "x": np.ascontiguousarray(np.asarray(inputs[b], dtype=np.float32)),
                "w1": w1,
                "w2a": w2a,
                "selo": selo,
                "mrow": np.ascontiguousarray(m[None, :]),
                "ones1": ones1,
                "ident": ident,
                "c0": c0,
            }
        )
    br = run_bass_kernel_spmd(nc, in_maps, core_ids=list(range(NCORES)), trace=trace)
    out = np.stack([br.results[b]["out"] for b in range(NCORES)], axis=0)
    return out, br


def kernel(inputs, weight1, weight2, mask):
    out, _ = _run(inputs, weight1, weight2, mask)
    return out
